# revision 14
# baseline (speedup 1.0000x reference)
"""Trainium2 Bass kernel for nn_AtteNet (8 NeuronCores, SPMD).

Strategy:
  - L1 (2048x20000 @ 20000x800): contraction(K)-sharded across 8 cores
    (2500 rows each). Partial products reduced with batch-chunked
    AllReduce (4 chunks of [800, 512]) pipelined under the L1 matmuls;
    after the last chunk every core holds the full pre-BN y1.
  - Everything downstream (BN1..BN4, L2-L4, attention, hazard) is
    replicated full-batch on every core -- no further collectives, no
    core-dependent addressing. Host reads core 0's outputs.
  - BatchNorm in [features(partitions), batch(free)] layout via
    bn_stats/bn_aggr; Linear biases b1..b4 skipped (they cancel in BN);
    inv_std via DVE bit-hack Newton (no ACT table switch).
  - All matmuls bf16 (inputs rounded on host); BN/activation math fp32.
"""
import os
import sys

sys.path.insert(0, "/opt/trn_rl_repo")

import numpy as np
import ml_dtypes

import concourse.bass as bass
import concourse.mybir as mybir
from concourse import tile, bacc
from concourse.bass_utils import run_bass_kernel_spmd

NCORES = 8
CORE_IDS = list(range(NCORES))
RG = [CORE_IDS]

B = 2048
MR = 20000
KSH = MR // NCORES          # 2500 contraction rows per core
F1, F2, F2P, F3, F4 = 800, 500, 512, 200, 128
NCH = B // 512              # 4 batch chunks of 512
EPS = 1e-5

bf16 = mybir.dt.bfloat16
f32 = mybir.dt.float32
i32 = mybir.dt.int32
AF = mybir.ActivationFunctionType
ALU = mybir.AluOpType

LAST_EXEC_NS = None         # set when KERNEL_TRACE=1


def _chunks(total, step=128):
    return [(i, min(step, total - i)) for i in range(0, total, step)]


KC1 = _chunks(KSH)          # 20 chunks (19x128 + 68) over 2500
KCF1 = _chunks(F1)          # 7 chunks (6x128 + 32) over 800
MT1 = KCF1                  # L1 output feature tiles
MT2 = _chunks(F2P)          # 4x128
MT3 = _chunks(F3)           # 128 + 72
KC4 = MT3


def _rsqrt(nc, pool, var_ap, P, name, n=1):
    """inv_std[P, n] = 1/sqrt(var + EPS) on DVE only (bit-hack + Newton)."""
    v = pool.tile([P, n], f32, name=f"rsq_v_{name}", tag=f"rsq_v_{name}")
    nc.vector.tensor_scalar_add(v[:, :], var_ap, EPS)
    magic = pool.tile([P, n], i32, name=f"rsq_m_{name}", tag=f"rsq_m_{name}")
    nc.vector.memset(magic[:, :], 0x5F3759DF)
    y = pool.tile([P, n], f32, name=f"rsq_y_{name}", tag=f"rsq_y_{name}")
    hi = pool.tile([P, n], i32, name=f"rsq_h_{name}", tag=f"rsq_h_{name}")
    nc.vector.tensor_scalar(hi[:, :], v[:, :].bitcast(i32), 1, None,
                            op0=ALU.logical_shift_right)
    nc.vector.tensor_tensor(y[:, :].bitcast(i32), magic[:, :], hi[:, :],
                            op=ALU.subtract)
    t = pool.tile([P, n], f32, name=f"rsq_t_{name}", tag=f"rsq_t_{name}")
    for _ in range(3):
        nc.vector.tensor_tensor(t[:, :], y[:, :], y[:, :], op=ALU.mult)
        nc.vector.tensor_tensor(t[:, :], t[:, :], v[:, :], op=ALU.mult)
        nc.vector.tensor_scalar(t[:, :], t[:, :], -0.5, 1.5,
                                op0=ALU.mult, op1=ALU.add)
        nc.vector.tensor_tensor(y[:, :], y[:, :], t[:, :], op=ALU.mult)
    return y


def _bn_coeffs(nc, pool, mv_mean, mv_var, g_ap, be_ap, P, name, n=1):
    """scale[P,n] = g*inv_std ; bias[P,n] = be - mean*scale."""
    inv = _rsqrt(nc, pool, mv_var, P, name, n)
    sc = pool.tile([P, n], f32, name=f"bn_s_{name}", tag=f"bn_s_{name}")
    nc.vector.tensor_tensor(sc[:, :], g_ap, inv[:, :], op=ALU.mult)
    bi = pool.tile([P, n], f32, name=f"bn_b_{name}", tag=f"bn_b_{name}")
    nc.vector.tensor_tensor(bi[:, :], mv_mean, sc[:, :], op=ALU.mult)
    nc.vector.tensor_tensor(bi[:, :], be_ap, bi[:, :], op=ALU.subtract)
    return sc, bi


def build():
    nc = bacc.Bacc("TRN2", target_bir_lowering=False, debug=False,
                   num_devices=NCORES)

    def din(name, shape, dt=bf16):
        return nc.dram_tensor(name, shape, dt, kind="ExternalInput").ap()

    xT = din("xT", [KSH, B])
    w1 = din("w1", [KSH, F1])
    w2 = din("w2", [F1, F2P])
    w3 = din("w3", [F2P, F3])
    w4 = din("w4", [F3, F4])
    a0T = din("a0T", [128, 128])
    a1T = din("a1T", [128, 128])
    wc = din("wc", [24, 128])
    clinT = din("clinT", [24, B])
    wh = din("wh", [128, 1])
    g1 = din("g1", [128, len(KCF1)], f32)    # 800 padded to 896, packed
    be1 = din("be1", [128, len(KCF1)], f32)
    g2 = din("g2", [128, len(MT2)], f32)
    be2 = din("be2", [128, len(MT2)], f32)
    g3 = din("g3", [128, len(MT3)], f32)
    be3 = din("be3", [128, len(MT3)], f32)
    g4 = din("g4", [F4, 1], f32)
    be4 = din("be4", [F4, 1], f32)
    bc = din("bc", [128, 1], f32)
    bh = din("bh", [1, 1], f32)

    out_v = nc.dram_tensor("out_v", [2, F4, B], f32, kind="ExternalOutput").ap()
    out_h = nc.dram_tensor("out_h", [1, B], f32, kind="ExternalOutput").ap()

    from contextlib import ExitStack

    with tile.TileContext(nc) as tc, ExitStack() as es:
        cpool = es.enter_context(tc.tile_pool(name="c", bufs=1))
        psum = es.enter_context(tc.tile_pool(name="psx", bufs=8, space="PSUM"))
        spool = es.enter_context(tc.tile_pool(name="s", bufs=1))
        opool = es.enter_context(tc.tile_pool(name="o", bufs=6))
        dram = es.enter_context(tc.tile_pool(name="d", bufs=4, space="DRAM"))
        hpool = es.enter_context(tc.tile_pool(name="h", bufs=1))

        # ---- constants to SBUF (tiny) ----
        def cload(ap_in, P, W_, dt, name):
            t = cpool.tile([P, W_], dt, name=f"c_{name}", tag=f"c_{name}")
            nc.sync.dma_start(t[:, :], ap_in)
            return t

        g1_s = cload(g1[:, :], 128, len(KCF1), f32, "g1")
        be1_s = cload(be1[:, :], 128, len(KCF1), f32, "be1")
        g2_s = cload(g2[:, :], 128, len(MT2), f32, "g2")
        be2_s = cload(be2[:, :], 128, len(MT2), f32, "be2")
        g3_s = cload(g3[:, :], 128, len(MT3), f32, "g3")
        be3_s = cload(be3[:, :], 128, len(MT3), f32, "be3")
        g4_s = cload(g4[:, :], F4, 1, f32, "g4")
        be4_s = cload(be4[:, :], F4, 1, f32, "be4")
        bc_s = cload(bc[:, :], 128, 1, f32, "bc")
        bh_s = cload(bh[:, :], 1, 1, f32, "bh")
        wh_s = cload(wh[:, :], 128, 1, bf16, "wh")
        a0T_s = cload(a0T[:, :], 128, 128, bf16, "a0T")
        a1T_s = cload(a1T[:, :], 128, 128, bf16, "a1T")
        wc_s = cload(wc[:, :], 24, 128, bf16, "wc")

        # ---- L1: partial = w1_c.T @ xT_c; batch-chunked AllReduce ----
        y1_s = hpool.tile([128, len(KCF1) * B], bf16, name="y1_s")
        stats1 = spool.tile([128, len(KCF1), NCH, 6], f32, name="stats1")
        with tc.tile_pool(name="p1", bufs=1) as p1:
            w1_s = p1.tile([128, len(KC1) * F1], bf16, name="w1_s")
            xs0 = []
            for ki, (k0, kn) in enumerate(KC1):
                nc.sync.dma_start(w1_s[:kn, ki * F1:(ki + 1) * F1],
                                  w1[k0:k0 + kn, :])
                xc = p1.tile([128, 512], bf16, name="xc", tag="xc", bufs=24)
                nc.sync.dma_start(xc[:kn, :], xT[k0:k0 + kn, 0:512])
                xs0.append(xc)
            for j in range(NCH):
                if j == 0:
                    xs = xs0
                else:
                    xs = []
                    for ki, (k0, kn) in enumerate(KC1):
                        xc = p1.tile([128, 512], bf16, name="xc", tag="xc",
                                     bufs=24)
                        nc.sync.dma_start(
                            xc[:kn, :], xT[k0:k0 + kn, j * 512:(j + 1) * 512])
                        xs.append(xc)
                part = dram.tile([F1, 512], bf16, name=f"part{j}", tag="part",
                                 bufs=4)
                for mi, (m0, mn) in enumerate(MT1):
                    ps = psum.tile([128, 512], f32, name=f"ps1_{j}_{mi}",
                                   tag="ps")
                    for ki, (k0, kn) in enumerate(KC1):
                        nc.tensor.matmul(
                            ps[:mn, :],
                            w1_s[:kn, ki * F1 + m0: ki * F1 + m0 + mn],
                            xs[ki][:kn, :],
                            start=(ki == 0), stop=(ki == len(KC1) - 1))
                    ob = opool.tile([128, 512], bf16, name="ob", tag="ob")
                    nc.vector.tensor_copy(ob[:mn, :], ps[:mn, :])
                    nc.sync.dma_start(part[m0:m0 + mn, :], ob[:mn, :])
                yd = dram.tile([F1, 512], bf16, name=f"y1d{j}", tag="y1d",
                               bufs=4, addr_space="Shared")
                nc.gpsimd.collective_compute(
                    "AllReduce", ALU.add, replica_groups=RG,
                    ins=[part[:, :].opt()], outs=[yd[:, :].opt()])
                for kk, (k0, kn) in enumerate(KCF1):
                    nc.sync.dma_start(
                        y1_s[:kn, kk * B + j * 512: kk * B + (j + 1) * 512],
                        yd[k0:k0 + kn, :])
                    nc.vector.bn_stats(
                        stats1[:, kk, j, :],
                        y1_s[:, kk * B + j * 512: kk * B + (j + 1) * 512])

        # ---- clinical path (independent; fills gaps) ----
        clinT_s = hpool.tile([24, B], bf16, name="clinT_s")
        nc.sync.dma_start(clinT_s[:, :], clinT[:, :])
        clinf = hpool.tile([128, B], f32, name="clinf")
        clinb = hpool.tile([128, B], bf16, name="clinb")
        t1f = hpool.tile([128, B], bf16, name="t1f")
        for n in range(NCH):
            pc = psum.tile([128, 512], f32, name="ps_clin", tag="ps")
            nc.tensor.matmul(pc[:, :], wc_s[:, :],
                             clinT_s[:, n * 512:(n + 1) * 512],
                             start=True, stop=True)
            nc.scalar.activation(clinf[:, n * 512:(n + 1) * 512], pc[:, :],
                                 AF.Sigmoid, bias=bc_s[:, 0:1], scale=1.0)
            nc.vector.tensor_copy(clinb[:, n * 512:(n + 1) * 512],
                                  clinf[:, n * 512:(n + 1) * 512])
            pa1 = psum.tile([128, 512], f32, name="ps_a1", tag="ps")
            nc.tensor.matmul(pa1[:, :], a1T_s[:, :],
                             clinb[:, n * 512:(n + 1) * 512],
                             start=True, stop=True)
            nc.scalar.activation(t1f[:, n * 512:(n + 1) * 512], pa1[:, :],
                                 AF.Tanh)
        nc.sync.dma_start(out_v[1, :, :], clinf[:, :])

        # ---- small-weight prefetch for L2-L4 ----
        w2_s = hpool.tile([128, len(KCF1) * F2P], bf16, name="w2_s")
        for kk, (k0, kn) in enumerate(KCF1):
            nc.sync.dma_start(w2_s[:kn, kk * F2P:(kk + 1) * F2P],
                              w2[k0:k0 + kn, :])
        w3_s = hpool.tile([128, len(MT2) * F3], bf16, name="w3_s")
        for kk, (k0, kn) in enumerate(MT2):
            nc.sync.dma_start(w3_s[:kn, kk * F3:(kk + 1) * F3],
                              w3[k0:k0 + kn, :])
        w4_s = hpool.tile([128, len(KC4) * F4], bf16, name="w4_s")
        for kk, (k0, kn) in enumerate(KC4):
            nc.sync.dma_start(w4_s[:kn, kk * F4:(kk + 1) * F4],
                              w4[k0:k0 + kn, :])

        # ---- BN1 + sigmoid (replicated, in-place on y1_s) ----
        h1_s = y1_s
        mv1 = spool.tile([128, len(KCF1), 2], f32, name="mv1")
        for kk in range(len(KCF1)):
            nc.vector.bn_aggr(mv1[:, kk, :], stats1[:, kk, :, :])
        sc1, bi1 = _bn_coeffs(nc, spool, mv1[:, :, 0], mv1[:, :, 1],
                              g1_s[:, :], be1_s[:, :], 128, "bn1",
                              n=len(KCF1))
        for kk, (k0, kn) in enumerate(KCF1):
            nc.scalar.activation(h1_s[:kn, kk * B:(kk + 1) * B],
                                 y1_s[:kn, kk * B:(kk + 1) * B],
                                 AF.Sigmoid, bias=bi1[:kn, kk:kk + 1],
                                 scale=sc1[:kn, kk:kk + 1])

        # ---- L2: per-m, kk-inner accumulation (consumes h1 as it appears) --
        h2_s = hpool.tile([128, len(MT2) * B], bf16, name="h2_s")
        stats2 = spool.tile([128, len(MT2), NCH, 6], f32, name="stats2")
        mv2 = spool.tile([128, len(MT2), 2], f32, name="mv2")
        with tc.tile_pool(name="p2", bufs=1) as p2:
            for mi, (m0, mn) in enumerate(MT2):
                y2t = p2.tile([128, B], bf16, name="y2t", tag="y2", bufs=3)
                pss = [psum.tile([128, 512], f32, name=f"ps2_{mi}_{n}",
                                 tag="ps") for n in range(NCH)]
                for kk, (k0, kn) in enumerate(KCF1):
                    for n in range(NCH):
                        nc.tensor.matmul(
                            pss[n][:mn, :],
                            w2_s[:kn, kk * F2P + m0: kk * F2P + m0 + mn],
                            h1_s[:kn, kk * B + n * 512: kk * B + (n + 1) * 512],
                            start=(kk == 0), stop=(kk == len(KCF1) - 1))
                for n in range(NCH):
                    nc.vector.bn_stats(stats2[:, mi, n, :], pss[n][:, :])
                    nc.scalar.copy(y2t[:, n * 512:(n + 1) * 512], pss[n][:, :])
                nc.vector.bn_aggr(mv2[:, mi, :], stats2[:, mi, :, :])
                sc2, bi2 = _bn_coeffs(nc, spool, mv2[:, mi, 0:1],
                                      mv2[:, mi, 1:2], g2_s[:, mi:mi + 1],
                                      be2_s[:, mi:mi + 1], 128, f"bn2_{mi}")
                nc.scalar.activation(h2_s[:, mi * B:(mi + 1) * B], y2t[:, :],
                                     AF.Sigmoid, bias=bi2[:, 0:1],
                                     scale=sc2[:, 0:1])

        # ---- L3: kk-outer accumulation pipelines with BN2 applies ----
        h3_s = hpool.tile([128, len(MT3) * B], bf16, name="h3_s")
        with tc.tile_pool(name="p3", bufs=1) as p3:
            y3_s = p3.tile([128, len(MT3) * B], bf16, name="y3_s")
            stats3 = spool.tile([128, len(MT3), NCH, 6], f32, name="stats3")
            ps3 = [[psum.tile([128, 512], f32, name=f"ps3_{mi}_{n}", tag="ps")
                    for n in range(NCH)] for mi in range(len(MT3))]
            for kk, (k0, kn) in enumerate(MT2):
                for mi, (m0, mn) in enumerate(MT3):
                    for n in range(NCH):
                        nc.tensor.matmul(
                            ps3[mi][n][:mn, :],
                            w3_s[:kn, kk * F3 + m0: kk * F3 + m0 + mn],
                            h2_s[:kn, kk * B + n * 512: kk * B + (n + 1) * 512],
                            start=(kk == 0), stop=(kk == len(MT2) - 1))
            mv3 = spool.tile([128, len(MT3), 2], f32, name="mv3")
            for mi, (m0, mn) in enumerate(MT3):
                for n in range(NCH):
                    nc.vector.bn_stats(stats3[:, mi, n, :], ps3[mi][n][:, :])
                    nc.scalar.copy(
                        y3_s[:mn, mi * B + n * 512: mi * B + (n + 1) * 512],
                        ps3[mi][n][:mn, :])
                nc.vector.bn_aggr(mv3[:, mi, :], stats3[:, mi, :, :])
                sc3, bi3 = _bn_coeffs(nc, spool, mv3[:, mi, 0:1],
                                      mv3[:, mi, 1:2], g3_s[:, mi:mi + 1],
                                      be3_s[:, mi:mi + 1], 128, f"bn3_{mi}")
                nc.scalar.activation(h3_s[:mn, mi * B:(mi + 1) * B],
                                     y3_s[:mn, mi * B:(mi + 1) * B],
                                     AF.Sigmoid, bias=bi3[:mn, 0:1],
                                     scale=sc3[:mn, 0:1])

        # ---- L4 ----
        y4_s = hpool.tile([F4, B], bf16, name="y4_s")
        stats4 = spool.tile([F4, NCH * 6], f32, name="stats4")
        ps4 = [psum.tile([128, 512], f32, name=f"ps4_{n}", tag="ps")
               for n in range(NCH)]
        for kk, (k0, kn) in enumerate(KC4):
            for n in range(NCH):
                nc.tensor.matmul(
                    ps4[n][:, :],
                    w4_s[:kn, kk * F4:(kk + 1) * F4],
                    h3_s[:kn, kk * B + n * 512: kk * B + (n + 1) * 512],
                    start=(kk == 0), stop=(kk == len(KC4) - 1))
        for n in range(NCH):
            nc.vector.bn_stats(stats4[:, n * 6:(n + 1) * 6], ps4[n][:, :])
            nc.vector.tensor_copy(y4_s[:, n * 512:(n + 1) * 512], ps4[n][:, :])

        # ---- BN4 + sigmoid; attention + hazard (chunked, bf16 elemwise) ----
        mv4 = spool.tile([F4, 2], f32, name="mv4")
        nc.vector.bn_aggr(mv4[:, :], stats4[:, :])
        sc4, bi4 = _bn_coeffs(nc, spool, mv4[:, 0:1], mv4[:, 1:2],
                              g4_s[:, 0:1], be4_s[:, 0:1], F4, "bn4")
        h4f = hpool.tile([128, B], f32, name="h4f")
        h4b = hpool.tile([128, B], bf16, name="h4b")
        t0f = hpool.tile([128, B], bf16, name="t0f")
        s0f = hpool.tile([128, B], bf16, name="s0f")
        e = hpool.tile([128, B], bf16, name="e_hc")
        cb = hpool.tile([128, B], bf16, name="cb")
        oh = hpool.tile([1, B], f32, name="oh")
        for n in range(NCH):
            sl = slice(n * 512, (n + 1) * 512)
            nc.scalar.activation(h4f[:, sl], y4_s[:, sl], AF.Sigmoid,
                                 bias=bi4[:, 0:1], scale=sc4[:, 0:1])
            nc.vector.tensor_copy(h4b[:, sl], h4f[:, sl])
            pa0 = psum.tile([128, 512], f32, name="ps_a0", tag="ps")
            nc.tensor.matmul(pa0[:, :], a0T_s[:, :], h4b[:, sl],
                             start=True, stop=True)
            nc.scalar.activation(t0f[:, sl], pa0[:, :], AF.Tanh)
            nc.vector.tensor_tensor(t0f[:, sl], t0f[:, sl], t1f[:, sl],
                                    op=ALU.subtract)
            nc.scalar.activation(s0f[:, sl], t0f[:, sl], AF.Sigmoid)
            nc.vector.tensor_tensor(e[:, sl], h4f[:, sl], clinf[:, sl],
                                    op=ALU.subtract)
            nc.vector.tensor_tensor(e[:, sl], s0f[:, sl], e[:, sl],
                                    op=ALU.mult)
            nc.vector.tensor_tensor(cb[:, sl], clinf[:, sl], e[:, sl],
                                    op=ALU.add)
            ph = psum.tile([1, 512], f32, name="ps_h", tag="ps")
            nc.tensor.matmul(ph[:, :], wh_s[:, 0:1], cb[:, sl],
                             start=True, stop=True)
            nc.vector.tensor_scalar(oh[:, sl], ph[:, :], bh_s[0:1, 0:1],
                                    None, op0=ALU.add)
        nc.sync.dma_start(out_v[0, :, :], h4f[:, :])
        nc.sync.dma_start(out_h[:, :], oh[:, :])

    nc.compile()
    return nc


_NC_CACHE = None


def _get_nc():
    global _NC_CACHE
    if _NC_CACHE is None:
        _NC_CACHE = build()
    return _NC_CACHE


def _pack_cols(vec, ntiles, fill):
    """[N] -> [128, ntiles]; column i = vec[i*128:(i+1)*128] (padded)."""
    out = np.full((ntiles * 128,), fill, np.float32)
    out[:len(vec)] = vec
    return np.ascontiguousarray(out.reshape(ntiles, 128).T)


def kernel(**inputs):
    global LAST_EXEC_NS
    f = np.float32
    bf = ml_dtypes.bfloat16
    mrna = np.asarray(inputs["mrna"], f)
    clin_cat = np.asarray(inputs["clin_cat"])
    clin_cont = np.asarray(inputs["clin_cont"], f)
    W = np.asarray(inputs["W"], f)
    w1f = np.asarray(inputs["w1"], f)
    w2f = np.asarray(inputs["w2"], f)
    w3f = np.asarray(inputs["w3"], f)
    w4f = np.asarray(inputs["w4"], f)
    embs = [np.asarray(inputs[f"emb{i}"], f) for i in range(4)]

    w2p = np.zeros((F1, F2P), f)
    w2p[:, :F2] = w2f
    w3p = np.zeros((F2P, F3), f)
    w3p[:F2, :] = w3f
    g2p = np.ones((F2P,), f)
    g2p[:F2] = np.asarray(inputs["g2"], f)
    be2p = np.zeros((F2P,), f)
    be2p[:F2] = np.asarray(inputs["be2"], f)

    clin_in = np.concatenate(
        [embs[i][clin_cat[:, i]] for i in range(4)] + [clin_cont], axis=1)

    shared = {
        "w2": w2p.astype(bf), "w3": w3p.astype(bf), "w4": w4f.astype(bf),
        "a0T": np.ascontiguousarray(W[0].T).astype(bf),
        "a1T": np.ascontiguousarray(W[1].T).astype(bf),
        "wc": np.asarray(inputs["wc"], f).astype(bf),
        "clinT": np.ascontiguousarray(clin_in.T).astype(bf),
        "wh": np.asarray(inputs["wh"], f).reshape(128, 1).astype(bf),
        "g1": _pack_cols(np.asarray(inputs["g1"], f), len(KCF1), 1.0),
        "be1": _pack_cols(np.asarray(inputs["be1"], f), len(KCF1), 0.0),
        "g2": _pack_cols(g2p, len(MT2), 1.0),
        "be2": _pack_cols(be2p, len(MT2), 0.0),
        "g3": _pack_cols(np.asarray(inputs["g3"], f), len(MT3), 1.0),
        "be3": _pack_cols(np.asarray(inputs["be3"], f), len(MT3), 0.0),
        "g4": np.asarray(inputs["g4"], f).reshape(-1, 1),
        "be4": np.asarray(inputs["be4"], f).reshape(-1, 1),
        "bc": np.asarray(inputs["bc"], f).reshape(-1, 1),
        "bh": np.asarray(inputs["bh"], f).reshape(1, 1),
    }

    in_maps = []
    for c in range(NCORES):
        m = dict(shared)
        m["xT"] = np.ascontiguousarray(
            mrna[:, c * KSH:(c + 1) * KSH].T).astype(bf)
        m["w1"] = w1f[c * KSH:(c + 1) * KSH, :].astype(bf)
        in_maps.append(m)

    nc = _get_nc()
    trace = bool(int(os.environ.get("KERNEL_TRACE", "0")))
    if trace:
        _install_trace_hook()
    res = run_bass_kernel_spmd(nc, in_maps, CORE_IDS, trace=trace)
    LAST_EXEC_NS = res.exec_time_ns

    ov = res.results[0]["out_v"].astype(f)      # [2, 128, 2048]
    ohv = res.results[0]["out_h"].astype(f)     # [1, 2048]
    v = np.ascontiguousarray(np.transpose(ov, (2, 0, 1)))[..., None]
    hazard = np.ascontiguousarray(ohv.T)        # [2048, 1]
    return hazard, v


def _install_trace_hook():
    import contextlib
    import ctypes
    import types

    if "antenv.axon_hooks" in sys.modules:
        return
    lib = ctypes.CDLL("/opt/axon/libaxon_pjrt.so")
    hook = None
    if hasattr(lib, "axon_start_nrt_profile"):
        lib.axon_start_nrt_profile.argtypes = [
            ctypes.POINTER(ctypes.c_int64), ctypes.c_size_t]
        lib.axon_start_nrt_profile.restype = ctypes.c_int64
        lib.axon_stop_nrt_profile.argtypes = [ctypes.c_char_p]
        lib.axon_stop_nrt_profile.restype = ctypes.c_int64

        @contextlib.contextmanager
        def hook(output_dir, device_ids):
            import jax
            jax.devices()
            if device_ids:
                ids = (ctypes.c_int64 * len(device_ids))(*device_ids)
                rc = lib.axon_start_nrt_profile(ids, len(device_ids))
            else:
                rc = lib.axon_start_nrt_profile(None, 0)
            if rc != 0:
                raise RuntimeError(f"start_nrt_profile rc={rc}")
            try:
                yield
            finally:
                n = lib.axon_stop_nrt_profile(str(output_dir).encode())
                print(f"profile: {n} files -> {output_dir}", file=sys.stderr)

    mod = types.ModuleType("antenv.axon_hooks")
    mod.get_axon_ntff_profile_hook = lambda: hook
    mod.set_axon_ntff_profile_hook = lambda h: None
    sys.modules["antenv.axon_hooks"] = mod


# revision 15
# speedup vs baseline: 1.0079x; 1.0079x over previous
"""Trainium2 Bass kernel for nn_AtteNet (8 NeuronCores, SPMD).

Strategy:
  - L1 (2048x20000 @ 20000x800): contraction(K)-sharded across 8 cores
    (2500 rows each). Partial products reduced with batch-chunked
    AllReduce (4 chunks of [800, 512]) pipelined under the L1 matmuls;
    after the last chunk every core holds the full pre-BN y1.
  - Everything downstream (BN1..BN4, L2-L4, attention, hazard) is
    replicated full-batch on every core -- no further collectives, no
    core-dependent addressing. Host reads core 0's outputs.
  - BatchNorm in [features(partitions), batch(free)] layout via
    bn_stats/bn_aggr; Linear biases b1..b4 skipped (they cancel in BN);
    inv_std via DVE bit-hack Newton (no ACT table switch).
  - All matmuls bf16 (inputs rounded on host); BN/activation math fp32.
"""
import os
import sys

sys.path.insert(0, "/opt/trn_rl_repo")

import numpy as np
import ml_dtypes

import concourse.bass as bass
import concourse.mybir as mybir
from concourse import tile, bacc
from concourse.bass_utils import run_bass_kernel_spmd

NCORES = 8
CORE_IDS = list(range(NCORES))
RG = [CORE_IDS]

B = 2048
MR = 20000
KSH = MR // NCORES          # 2500 contraction rows per core
F1, F2, F2P, F3, F4 = 800, 500, 512, 200, 128
NCH = B // 512              # 4 batch chunks of 512
EPS = 1e-5

bf16 = mybir.dt.bfloat16
f32 = mybir.dt.float32
i32 = mybir.dt.int32
AF = mybir.ActivationFunctionType
ALU = mybir.AluOpType

LAST_EXEC_NS = None         # set when KERNEL_TRACE=1


def _chunks(total, step=128):
    return [(i, min(step, total - i)) for i in range(0, total, step)]


KC1 = _chunks(KSH)          # 20 chunks (19x128 + 68) over 2500
KCF1 = _chunks(F1)          # 7 chunks (6x128 + 32) over 800
MT1 = KCF1                  # L1 output feature tiles
MT2 = _chunks(F2P)          # 4x128
MT3 = _chunks(F3)           # 128 + 72
KC4 = MT3


def _rsqrt(nc, pool, var_ap, P, name, n=1):
    """inv_std[P, n] = 1/sqrt(var + EPS) on DVE only (bit-hack + Newton)."""
    v = pool.tile([P, n], f32, name=f"rsq_v_{name}", tag=f"rsq_v_{name}")
    nc.vector.tensor_scalar_add(v[:, :], var_ap, EPS)
    magic = pool.tile([P, n], i32, name=f"rsq_m_{name}", tag=f"rsq_m_{name}")
    nc.vector.memset(magic[:, :], 0x5F3759DF)
    y = pool.tile([P, n], f32, name=f"rsq_y_{name}", tag=f"rsq_y_{name}")
    hi = pool.tile([P, n], i32, name=f"rsq_h_{name}", tag=f"rsq_h_{name}")
    nc.vector.tensor_scalar(hi[:, :], v[:, :].bitcast(i32), 1, None,
                            op0=ALU.logical_shift_right)
    nc.vector.tensor_tensor(y[:, :].bitcast(i32), magic[:, :], hi[:, :],
                            op=ALU.subtract)
    t = pool.tile([P, n], f32, name=f"rsq_t_{name}", tag=f"rsq_t_{name}")
    for _ in range(2):
        nc.vector.tensor_tensor(t[:, :], y[:, :], y[:, :], op=ALU.mult)
        nc.vector.tensor_tensor(t[:, :], t[:, :], v[:, :], op=ALU.mult)
        nc.vector.tensor_scalar(t[:, :], t[:, :], -0.5, 1.5,
                                op0=ALU.mult, op1=ALU.add)
        nc.vector.tensor_tensor(y[:, :], y[:, :], t[:, :], op=ALU.mult)
    return y


def _bn_coeffs(nc, pool, mv_mean, mv_var, g_ap, be_ap, P, name, n=1):
    """scale[P,n] = g*inv_std ; bias[P,n] = be - mean*scale."""
    inv = _rsqrt(nc, pool, mv_var, P, name, n)
    sc = pool.tile([P, n], f32, name=f"bn_s_{name}", tag=f"bn_s_{name}")
    nc.vector.tensor_tensor(sc[:, :], g_ap, inv[:, :], op=ALU.mult)
    bi = pool.tile([P, n], f32, name=f"bn_b_{name}", tag=f"bn_b_{name}")
    nc.vector.tensor_tensor(bi[:, :], mv_mean, sc[:, :], op=ALU.mult)
    nc.vector.tensor_tensor(bi[:, :], be_ap, bi[:, :], op=ALU.subtract)
    return sc, bi


def build():
    nc = bacc.Bacc("TRN2", target_bir_lowering=False, debug=False,
                   num_devices=NCORES)

    def din(name, shape, dt=bf16):
        return nc.dram_tensor(name, shape, dt, kind="ExternalInput").ap()

    xT = din("xT", [KSH, B])
    w1 = din("w1", [KSH, F1])
    w2 = din("w2", [F1, F2P])
    w3 = din("w3", [F2P, F3])
    w4 = din("w4", [F3, F4])
    a0T = din("a0T", [128, 128])
    a1T = din("a1T", [128, 128])
    wc = din("wc", [24, 128])
    clinT = din("clinT", [24, B])
    wh = din("wh", [128, 1])
    g1 = din("g1", [F1 // NCORES, 1], f32)   # per-core feature shard
    be1 = din("be1", [F1 // NCORES, 1], f32)
    g2 = din("g2", [128, len(MT2)], f32)
    be2 = din("be2", [128, len(MT2)], f32)
    g3 = din("g3", [128, len(MT3)], f32)
    be3 = din("be3", [128, len(MT3)], f32)
    g4 = din("g4", [F4, 1], f32)
    be4 = din("be4", [F4, 1], f32)
    bc = din("bc", [128, 1], f32)
    bh = din("bh", [1, 1], f32)

    out_v = nc.dram_tensor("out_v", [2, F4, B], f32, kind="ExternalOutput").ap()
    out_h = nc.dram_tensor("out_h", [1, B], f32, kind="ExternalOutput").ap()

    from contextlib import ExitStack

    with tile.TileContext(nc) as tc, ExitStack() as es:
        cpool = es.enter_context(tc.tile_pool(name="c", bufs=1))
        psum = es.enter_context(tc.tile_pool(name="psx", bufs=8, space="PSUM"))
        spool = es.enter_context(tc.tile_pool(name="s", bufs=1))
        opool = es.enter_context(tc.tile_pool(name="o", bufs=6))
        dram = es.enter_context(tc.tile_pool(name="d", bufs=4, space="DRAM"))
        hpool = es.enter_context(tc.tile_pool(name="h", bufs=1))

        # ---- constants to SBUF (tiny) ----
        def cload(ap_in, P, W_, dt, name):
            t = cpool.tile([P, W_], dt, name=f"c_{name}", tag=f"c_{name}")
            nc.sync.dma_start(t[:, :], ap_in)
            return t

        g1_s = cload(g1[:, :], F1 // NCORES, 1, f32, "g1")
        be1_s = cload(be1[:, :], F1 // NCORES, 1, f32, "be1")
        g2_s = cload(g2[:, :], 128, len(MT2), f32, "g2")
        be2_s = cload(be2[:, :], 128, len(MT2), f32, "be2")
        g3_s = cload(g3[:, :], 128, len(MT3), f32, "g3")
        be3_s = cload(be3[:, :], 128, len(MT3), f32, "be3")
        g4_s = cload(g4[:, :], F4, 1, f32, "g4")
        be4_s = cload(be4[:, :], F4, 1, f32, "be4")
        bc_s = cload(bc[:, :], 128, 1, f32, "bc")
        bh_s = cload(bh[:, :], 1, 1, f32, "bh")
        wh_s = cload(wh[:, :], 128, 1, bf16, "wh")
        a0T_s = cload(a0T[:, :], 128, 128, bf16, "a0T")
        a1T_s = cload(a1T[:, :], 128, 128, bf16, "a1T")
        wc_s = cload(wc[:, :], 24, 128, bf16, "wc")

        # ---- L1: partial = w1_c.T @ xT_c; batch-chunked ReduceScatter ----
        F1SH = F1 // NCORES
        y1_s = hpool.tile([F1SH, B], bf16, name="y1_s")
        stats1 = spool.tile([F1SH, NCH * 6], f32, name="stats1")
        h1c = hpool.tile([F1SH, B], bf16, name="h1c")
        ago = dram.tile([F1, B], bf16, name="ago", tag="ago", bufs=1,
                        addr_space="Shared")
        with tc.tile_pool(name="p1", bufs=1) as p1:
            w1_s = p1.tile([128, len(KC1) * F1], bf16, name="w1_s")
            xs0 = []
            # first chunks split into strips so several DMA queues fill them
            for ki, (k0, kn) in enumerate(KC1):
                if ki < 4:
                    for q in range(4):
                        nc.sync.dma_start(
                            w1_s[:kn, ki * F1 + q * 200: ki * F1 + (q + 1) * 200],
                            w1[k0:k0 + kn, q * 200:(q + 1) * 200])
                else:
                    nc.sync.dma_start(w1_s[:kn, ki * F1:(ki + 1) * F1],
                                      w1[k0:k0 + kn, :])
                xc = p1.tile([128, 512], bf16, name="xc", tag="xc", bufs=24)
                if ki < 4:
                    nc.sync.dma_start(xc[:kn, 0:256], xT[k0:k0 + kn, 0:256])
                    nc.sync.dma_start(xc[:kn, 256:512], xT[k0:k0 + kn, 256:512])
                else:
                    nc.sync.dma_start(xc[:kn, :], xT[k0:k0 + kn, 0:512])
                xs0.append(xc)
            for j in range(NCH):
                if j == 0:
                    xs = xs0
                else:
                    xs = []
                    for ki, (k0, kn) in enumerate(KC1):
                        xc = p1.tile([128, 512], bf16, name="xc", tag="xc",
                                     bufs=24)
                        nc.sync.dma_start(
                            xc[:kn, :], xT[k0:k0 + kn, j * 512:(j + 1) * 512])
                        xs.append(xc)
                part = dram.tile([F1, 512], bf16, name=f"part{j}", tag="part",
                                 bufs=4)
                for mi, (m0, mn) in enumerate(MT1):
                    ps = psum.tile([128, 512], f32, name=f"ps1_{j}_{mi}",
                                   tag="ps")
                    for ki, (k0, kn) in enumerate(KC1):
                        nc.tensor.matmul(
                            ps[:mn, :],
                            w1_s[:kn, ki * F1 + m0: ki * F1 + m0 + mn],
                            xs[ki][:kn, :],
                            start=(ki == 0), stop=(ki == len(KC1) - 1))
                    ob = opool.tile([128, 512], bf16, name="ob", tag="ob")
                    nc.vector.tensor_copy(ob[:mn, :], ps[:mn, :])
                    nc.sync.dma_start(part[m0:m0 + mn, :], ob[:mn, :])
                rs = dram.tile([F1SH, 512], bf16, name=f"rs{j}", tag="rs",
                               bufs=4)
                nc.gpsimd.collective_compute(
                    "ReduceScatter", ALU.add, replica_groups=RG,
                    ins=[part[:, :].opt()], outs=[rs[:, :].opt()])
                nc.sync.dma_start(y1_s[:, j * 512:(j + 1) * 512], rs[:, :])
                nc.vector.bn_stats(stats1[:, j * 6:(j + 1) * 6],
                                   y1_s[:, j * 512:(j + 1) * 512])

            # BN1 + sigmoid on own shard, then one AllGather of h1
            mv1 = spool.tile([F1SH, 2], f32, name="mv1")
            nc.vector.bn_aggr(mv1[:, :], stats1[:, :])
            sc1, bi1 = _bn_coeffs(nc, spool, mv1[:, 0:1], mv1[:, 1:2],
                                  g1_s[:, 0:1], be1_s[:, 0:1], F1SH, "bn1")
            agi = dram.tile([F1SH, B], bf16, name="agi", tag="agi", bufs=1)
            for j in range(NCH):
                nc.scalar.activation(h1c[:, j * 512:(j + 1) * 512],
                                     y1_s[:, j * 512:(j + 1) * 512],
                                     AF.Sigmoid, bias=bi1[:, 0:1],
                                     scale=sc1[:, 0:1])
                nc.sync.dma_start(agi[:, j * 512:(j + 1) * 512],
                                  h1c[:, j * 512:(j + 1) * 512])
            nc.gpsimd.collective_compute(
                "AllGather", ALU.bypass, replica_groups=RG,
                ins=[agi[:, :].opt()], outs=[ago[:, :].opt()])

        # ---- clinical path (independent; fills gaps) ----
        clinT_s = hpool.tile([24, B], bf16, name="clinT_s")
        nc.sync.dma_start(clinT_s[:, :], clinT[:, :])
        clinf = hpool.tile([128, B], f32, name="clinf")
        clinb = hpool.tile([128, B], bf16, name="clinb")
        t1f = hpool.tile([128, B], bf16, name="t1f")
        for n in range(NCH):
            pc = psum.tile([128, 512], f32, name="ps_clin", tag="ps")
            nc.tensor.matmul(pc[:, :], wc_s[:, :],
                             clinT_s[:, n * 512:(n + 1) * 512],
                             start=True, stop=True)
            nc.scalar.activation(clinf[:, n * 512:(n + 1) * 512], pc[:, :],
                                 AF.Sigmoid, bias=bc_s[:, 0:1], scale=1.0)
            nc.vector.tensor_copy(clinb[:, n * 512:(n + 1) * 512],
                                  clinf[:, n * 512:(n + 1) * 512])
            pa1 = psum.tile([128, 512], f32, name="ps_a1", tag="ps")
            nc.tensor.matmul(pa1[:, :], a1T_s[:, :],
                             clinb[:, n * 512:(n + 1) * 512],
                             start=True, stop=True)
            nc.scalar.activation(t1f[:, n * 512:(n + 1) * 512], pa1[:, :],
                                 AF.Tanh)
        nc.sync.dma_start(out_v[1, :, :], clinf[:, :])

        # ---- small-weight prefetch for L2-L4 ----
        w2_s = hpool.tile([128, len(KCF1) * F2P], bf16, name="w2_s")
        for kk, (k0, kn) in enumerate(KCF1):
            nc.sync.dma_start(w2_s[:kn, kk * F2P:(kk + 1) * F2P],
                              w2[k0:k0 + kn, :])
        w3_s = hpool.tile([128, len(MT2) * F3], bf16, name="w3_s")
        for kk, (k0, kn) in enumerate(MT2):
            nc.sync.dma_start(w3_s[:kn, kk * F3:(kk + 1) * F3],
                              w3[k0:k0 + kn, :])
        w4_s = hpool.tile([128, len(KC4) * F4], bf16, name="w4_s")
        for kk, (k0, kn) in enumerate(KC4):
            nc.sync.dma_start(w4_s[:kn, kk * F4:(kk + 1) * F4],
                              w4[k0:k0 + kn, :])

        # ---- h1 assembled from AllGather output, kk-chunked ----
        h1_s = hpool.tile([128, len(KCF1) * B], bf16, name="h1_s")
        for kk, (k0, kn) in enumerate(KCF1):
            nc.sync.dma_start(h1_s[:kn, kk * B:(kk + 1) * B], ago[k0:k0 + kn, :])

        # ---- L2: per-m, kk-inner accumulation (consumes h1 as it appears) --
        h2_s = hpool.tile([128, len(MT2) * B], bf16, name="h2_s")
        stats2 = spool.tile([128, len(MT2), NCH, 6], f32, name="stats2")
        mv2 = spool.tile([128, len(MT2), 2], f32, name="mv2")
        with tc.tile_pool(name="p2", bufs=1) as p2:
            for mi, (m0, mn) in enumerate(MT2):
                y2t = p2.tile([128, B], bf16, name="y2t", tag="y2", bufs=3)
                pss = [psum.tile([128, 512], f32, name=f"ps2_{mi}_{n}",
                                 tag="ps") for n in range(NCH)]
                for kk, (k0, kn) in enumerate(KCF1):
                    for n in range(NCH):
                        nc.tensor.matmul(
                            pss[n][:mn, :],
                            w2_s[:kn, kk * F2P + m0: kk * F2P + m0 + mn],
                            h1_s[:kn, kk * B + n * 512: kk * B + (n + 1) * 512],
                            start=(kk == 0), stop=(kk == len(KCF1) - 1))
                for n in range(NCH):
                    nc.vector.bn_stats(stats2[:, mi, n, :], pss[n][:, :])
                    nc.scalar.copy(y2t[:, n * 512:(n + 1) * 512], pss[n][:, :])
                nc.vector.bn_aggr(mv2[:, mi, :], stats2[:, mi, :, :])
                sc2, bi2 = _bn_coeffs(nc, spool, mv2[:, mi, 0:1],
                                      mv2[:, mi, 1:2], g2_s[:, mi:mi + 1],
                                      be2_s[:, mi:mi + 1], 128, f"bn2_{mi}")
                nc.scalar.activation(h2_s[:, mi * B:(mi + 1) * B], y2t[:, :],
                                     AF.Sigmoid, bias=bi2[:, 0:1],
                                     scale=sc2[:, 0:1])

        # ---- L3: kk-outer accumulation pipelines with BN2 applies ----
        h3_s = hpool.tile([128, len(MT3) * B], bf16, name="h3_s")
        with tc.tile_pool(name="p3", bufs=1) as p3:
            y3_s = p3.tile([128, len(MT3) * B], bf16, name="y3_s")
            stats3 = spool.tile([128, len(MT3), NCH, 6], f32, name="stats3")
            ps3 = [[psum.tile([128, 512], f32, name=f"ps3_{mi}_{n}", tag="ps")
                    for n in range(NCH)] for mi in range(len(MT3))]
            for kk, (k0, kn) in enumerate(MT2):
                for mi, (m0, mn) in enumerate(MT3):
                    for n in range(NCH):
                        nc.tensor.matmul(
                            ps3[mi][n][:mn, :],
                            w3_s[:kn, kk * F3 + m0: kk * F3 + m0 + mn],
                            h2_s[:kn, kk * B + n * 512: kk * B + (n + 1) * 512],
                            start=(kk == 0), stop=(kk == len(MT2) - 1))
            mv3 = spool.tile([128, len(MT3), 2], f32, name="mv3")
            for mi, (m0, mn) in enumerate(MT3):
                for n in range(NCH):
                    nc.vector.bn_stats(stats3[:, mi, n, :], ps3[mi][n][:, :])
                    nc.scalar.copy(
                        y3_s[:mn, mi * B + n * 512: mi * B + (n + 1) * 512],
                        ps3[mi][n][:mn, :])
                nc.vector.bn_aggr(mv3[:, mi, :], stats3[:, mi, :, :])
                sc3, bi3 = _bn_coeffs(nc, spool, mv3[:, mi, 0:1],
                                      mv3[:, mi, 1:2], g3_s[:, mi:mi + 1],
                                      be3_s[:, mi:mi + 1], 128, f"bn3_{mi}")
                nc.scalar.activation(h3_s[:mn, mi * B:(mi + 1) * B],
                                     y3_s[:mn, mi * B:(mi + 1) * B],
                                     AF.Sigmoid, bias=bi3[:mn, 0:1],
                                     scale=sc3[:mn, 0:1])

        # ---- L4 ----
        y4_s = hpool.tile([F4, B], bf16, name="y4_s")
        stats4 = spool.tile([F4, NCH * 6], f32, name="stats4")
        ps4 = [psum.tile([128, 512], f32, name=f"ps4_{n}", tag="ps")
               for n in range(NCH)]
        for kk, (k0, kn) in enumerate(KC4):
            for n in range(NCH):
                nc.tensor.matmul(
                    ps4[n][:, :],
                    w4_s[:kn, kk * F4:(kk + 1) * F4],
                    h3_s[:kn, kk * B + n * 512: kk * B + (n + 1) * 512],
                    start=(kk == 0), stop=(kk == len(KC4) - 1))
        for n in range(NCH):
            nc.vector.bn_stats(stats4[:, n * 6:(n + 1) * 6], ps4[n][:, :])
            nc.vector.tensor_copy(y4_s[:, n * 512:(n + 1) * 512], ps4[n][:, :])

        # ---- BN4 + sigmoid; attention + hazard (chunked, bf16 elemwise) ----
        mv4 = spool.tile([F4, 2], f32, name="mv4")
        nc.vector.bn_aggr(mv4[:, :], stats4[:, :])
        sc4, bi4 = _bn_coeffs(nc, spool, mv4[:, 0:1], mv4[:, 1:2],
                              g4_s[:, 0:1], be4_s[:, 0:1], F4, "bn4")
        h4b = hpool.tile([128, B], bf16, name="h4b")
        t0f = hpool.tile([128, B], bf16, name="t0f")
        s0f = hpool.tile([128, B], bf16, name="s0f")
        e = hpool.tile([128, B], bf16, name="e_hc")
        cb = hpool.tile([128, B], bf16, name="cb")
        oh = hpool.tile([1, B], f32, name="oh")
        for n in range(NCH):
            sl = slice(n * 512, (n + 1) * 512)
            nc.scalar.activation(h4b[:, sl], y4_s[:, sl], AF.Sigmoid,
                                 bias=bi4[:, 0:1], scale=sc4[:, 0:1])
            pa0 = psum.tile([128, 512], f32, name="ps_a0", tag="ps")
            nc.tensor.matmul(pa0[:, :], a0T_s[:, :], h4b[:, sl],
                             start=True, stop=True)
            nc.scalar.activation(t0f[:, sl], pa0[:, :], AF.Tanh)
            nc.vector.tensor_tensor(t0f[:, sl], t0f[:, sl], t1f[:, sl],
                                    op=ALU.subtract)
            nc.scalar.activation(s0f[:, sl], t0f[:, sl], AF.Sigmoid)
            nc.vector.tensor_tensor(e[:, sl], h4b[:, sl], clinb[:, sl],
                                    op=ALU.subtract)
            nc.vector.tensor_tensor(e[:, sl], s0f[:, sl], e[:, sl],
                                    op=ALU.mult)
            nc.vector.tensor_tensor(cb[:, sl], clinf[:, sl], e[:, sl],
                                    op=ALU.add)
            ph = psum.tile([1, 512], f32, name="ps_h", tag="ps")
            nc.tensor.matmul(ph[:, :], wh_s[:, 0:1], cb[:, sl],
                             start=True, stop=True)
            nc.vector.tensor_scalar(oh[:, sl], ph[:, :], bh_s[0:1, 0:1],
                                    None, op0=ALU.add)
        nc.gpsimd.dma_start(out_v[0, :, :], h4b[:, :])   # casts bf16 -> f32
        nc.sync.dma_start(out_h[:, :], oh[:, :])

    nc.compile()
    return nc


_NC_CACHE = None


def _get_nc():
    global _NC_CACHE
    if _NC_CACHE is None:
        _NC_CACHE = build()
    return _NC_CACHE


def _pack_cols(vec, ntiles, fill):
    """[N] -> [128, ntiles]; column i = vec[i*128:(i+1)*128] (padded)."""
    out = np.full((ntiles * 128,), fill, np.float32)
    out[:len(vec)] = vec
    return np.ascontiguousarray(out.reshape(ntiles, 128).T)


def kernel(**inputs):
    global LAST_EXEC_NS
    f = np.float32
    bf = ml_dtypes.bfloat16
    mrna = np.asarray(inputs["mrna"], f)
    clin_cat = np.asarray(inputs["clin_cat"])
    clin_cont = np.asarray(inputs["clin_cont"], f)
    W = np.asarray(inputs["W"], f)
    w1f = np.asarray(inputs["w1"], f)
    w2f = np.asarray(inputs["w2"], f)
    w3f = np.asarray(inputs["w3"], f)
    w4f = np.asarray(inputs["w4"], f)
    embs = [np.asarray(inputs[f"emb{i}"], f) for i in range(4)]
    g1f = np.asarray(inputs["g1"], f)
    be1f = np.asarray(inputs["be1"], f)

    w2p = np.zeros((F1, F2P), f)
    w2p[:, :F2] = w2f
    w3p = np.zeros((F2P, F3), f)
    w3p[:F2, :] = w3f
    g2p = np.ones((F2P,), f)
    g2p[:F2] = np.asarray(inputs["g2"], f)
    be2p = np.zeros((F2P,), f)
    be2p[:F2] = np.asarray(inputs["be2"], f)

    clin_in = np.concatenate(
        [embs[i][clin_cat[:, i]] for i in range(4)] + [clin_cont], axis=1)

    shared = {
        "w2": w2p.astype(bf), "w3": w3p.astype(bf), "w4": w4f.astype(bf),
        "a0T": np.ascontiguousarray(W[0].T).astype(bf),
        "a1T": np.ascontiguousarray(W[1].T).astype(bf),
        "wc": np.asarray(inputs["wc"], f).astype(bf),
        "clinT": np.ascontiguousarray(clin_in.T).astype(bf),
        "wh": np.asarray(inputs["wh"], f).reshape(128, 1).astype(bf),
        "g2": _pack_cols(g2p, len(MT2), 1.0),
        "be2": _pack_cols(be2p, len(MT2), 0.0),
        "g3": _pack_cols(np.asarray(inputs["g3"], f), len(MT3), 1.0),
        "be3": _pack_cols(np.asarray(inputs["be3"], f), len(MT3), 0.0),
        "g4": np.asarray(inputs["g4"], f).reshape(-1, 1),
        "be4": np.asarray(inputs["be4"], f).reshape(-1, 1),
        "bc": np.asarray(inputs["bc"], f).reshape(-1, 1),
        "bh": np.asarray(inputs["bh"], f).reshape(1, 1),
    }

    in_maps = []
    for c in range(NCORES):
        m = dict(shared)
        m["xT"] = np.ascontiguousarray(
            mrna[:, c * KSH:(c + 1) * KSH].T).astype(bf)
        m["w1"] = w1f[c * KSH:(c + 1) * KSH, :].astype(bf)
        m["g1"] = g1f[c * 100:(c + 1) * 100].reshape(-1, 1)
        m["be1"] = be1f[c * 100:(c + 1) * 100].reshape(-1, 1)
        in_maps.append(m)

    nc = _get_nc()
    trace = bool(int(os.environ.get("KERNEL_TRACE", "0")))
    if trace:
        _install_trace_hook()
    res = run_bass_kernel_spmd(nc, in_maps, CORE_IDS, trace=trace)
    LAST_EXEC_NS = res.exec_time_ns

    ov = res.results[0]["out_v"].astype(f)      # [2, 128, 2048]
    ohv = res.results[0]["out_h"].astype(f)     # [1, 2048]
    v = np.ascontiguousarray(np.transpose(ov, (2, 0, 1)))[..., None]
    hazard = np.ascontiguousarray(ohv.T)        # [2048, 1]
    return hazard, v


def _install_trace_hook():
    import contextlib
    import ctypes
    import types

    if "antenv.axon_hooks" in sys.modules:
        return
    lib = ctypes.CDLL("/opt/axon/libaxon_pjrt.so")
    hook = None
    if hasattr(lib, "axon_start_nrt_profile"):
        lib.axon_start_nrt_profile.argtypes = [
            ctypes.POINTER(ctypes.c_int64), ctypes.c_size_t]
        lib.axon_start_nrt_profile.restype = ctypes.c_int64
        lib.axon_stop_nrt_profile.argtypes = [ctypes.c_char_p]
        lib.axon_stop_nrt_profile.restype = ctypes.c_int64

        @contextlib.contextmanager
        def hook(output_dir, device_ids):
            import jax
            jax.devices()
            if device_ids:
                ids = (ctypes.c_int64 * len(device_ids))(*device_ids)
                rc = lib.axon_start_nrt_profile(ids, len(device_ids))
            else:
                rc = lib.axon_start_nrt_profile(None, 0)
            if rc != 0:
                raise RuntimeError(f"start_nrt_profile rc={rc}")
            try:
                yield
            finally:
                n = lib.axon_stop_nrt_profile(str(output_dir).encode())
                print(f"profile: {n} files -> {output_dir}", file=sys.stderr)

    mod = types.ModuleType("antenv.axon_hooks")
    mod.get_axon_ntff_profile_hook = lambda: hook
    mod.set_axon_ntff_profile_hook = lambda h: None
    sys.modules["antenv.axon_hooks"] = mod


# revision 17
# speedup vs baseline: 1.1183x; 1.1096x over previous
"""Trainium2 Bass kernel for nn_AtteNet (8 NeuronCores, SPMD).

Strategy:
  - L1 (2048x20000 @ 20000x800): contraction(K)-sharded across 8 cores
    (2500 rows each). Partial products reduced with batch-chunked
    AllReduce (4 chunks of [800, 512]) pipelined under the L1 matmuls;
    after the last chunk every core holds the full pre-BN y1.
  - Everything downstream (BN1..BN4, L2-L4, attention, hazard) is
    replicated full-batch on every core -- no further collectives, no
    core-dependent addressing. Host reads core 0's outputs.
  - BatchNorm in [features(partitions), batch(free)] layout via
    bn_stats/bn_aggr; Linear biases b1..b4 skipped (they cancel in BN);
    inv_std via DVE bit-hack Newton (no ACT table switch).
  - All matmuls bf16 (inputs rounded on host); BN/activation math fp32.
"""
import os
import sys

sys.path.insert(0, "/opt/trn_rl_repo")

import numpy as np
import ml_dtypes

import concourse.bass as bass
import concourse.mybir as mybir
from concourse import tile, bacc
from concourse.bass_utils import run_bass_kernel_spmd

NCORES = 8
CORE_IDS = list(range(NCORES))
RG = [CORE_IDS]

B = 2048
MR = 20000
KSH = MR // NCORES          # 2500 contraction rows per core
F1, F2, F2P, F3, F4 = 800, 500, 512, 200, 128
NCH = B // 512              # 4 batch chunks of 512
EPS = 1e-5

bf16 = mybir.dt.bfloat16
f32 = mybir.dt.float32
i32 = mybir.dt.int32
AF = mybir.ActivationFunctionType
ALU = mybir.AluOpType

LAST_EXEC_NS = None         # set when KERNEL_TRACE=1


def _chunks(total, step=128):
    return [(i, min(step, total - i)) for i in range(0, total, step)]


KC1 = _chunks(KSH)          # 20 chunks (19x128 + 68) over 2500
KCF1 = _chunks(F1)          # 7 chunks (6x128 + 32) over 800
MT1 = KCF1                  # L1 output feature tiles
MT2 = _chunks(F2P)          # 4x128
MT3 = _chunks(F3)           # 128 + 72
KC4 = MT3


def _rsqrt(nc, pool, var_ap, P, name, n=1):
    """inv_std[P, n] = 1/sqrt(var + EPS) on DVE only (bit-hack + Newton)."""
    v = pool.tile([P, n], f32, name=f"rsq_v_{name}", tag=f"rsq_v_{name}")
    nc.vector.tensor_scalar_add(v[:, :], var_ap, EPS)
    magic = pool.tile([P, n], i32, name=f"rsq_m_{name}", tag=f"rsq_m_{name}")
    nc.vector.memset(magic[:, :], 0x5F3759DF)
    y = pool.tile([P, n], f32, name=f"rsq_y_{name}", tag=f"rsq_y_{name}")
    hi = pool.tile([P, n], i32, name=f"rsq_h_{name}", tag=f"rsq_h_{name}")
    nc.vector.tensor_scalar(hi[:, :], v[:, :].bitcast(i32), 1, None,
                            op0=ALU.logical_shift_right)
    nc.vector.tensor_tensor(y[:, :].bitcast(i32), magic[:, :], hi[:, :],
                            op=ALU.subtract)
    t = pool.tile([P, n], f32, name=f"rsq_t_{name}", tag=f"rsq_t_{name}")
    for _ in range(2):
        nc.vector.tensor_tensor(t[:, :], y[:, :], y[:, :], op=ALU.mult)
        nc.vector.tensor_tensor(t[:, :], t[:, :], v[:, :], op=ALU.mult)
        nc.vector.tensor_scalar(t[:, :], t[:, :], -0.5, 1.5,
                                op0=ALU.mult, op1=ALU.add)
        nc.vector.tensor_tensor(y[:, :], y[:, :], t[:, :], op=ALU.mult)
    return y


def _bn_coeffs(nc, pool, mv_mean, mv_var, g_ap, be_ap, P, name, n=1):
    """scale[P,n] = g*inv_std ; bias[P,n] = be - mean*scale."""
    inv = _rsqrt(nc, pool, mv_var, P, name, n)
    sc = pool.tile([P, n], f32, name=f"bn_s_{name}", tag=f"bn_s_{name}")
    nc.vector.tensor_tensor(sc[:, :], g_ap, inv[:, :], op=ALU.mult)
    bi = pool.tile([P, n], f32, name=f"bn_b_{name}", tag=f"bn_b_{name}")
    nc.vector.tensor_tensor(bi[:, :], mv_mean, sc[:, :], op=ALU.mult)
    nc.vector.tensor_tensor(bi[:, :], be_ap, bi[:, :], op=ALU.subtract)
    return sc, bi


def build():
    nc = bacc.Bacc("TRN2", target_bir_lowering=False, debug=False,
                   num_devices=NCORES)

    def din(name, shape, dt=bf16):
        return nc.dram_tensor(name, shape, dt, kind="ExternalInput").ap()

    xT = din("xT", [KSH, B])
    w1 = din("w1", [KSH, F1])
    w2 = din("w2", [F1, F2P])
    w3 = din("w3", [F2P, F3])
    w4 = din("w4", [F3, F4])
    a0T = din("a0T", [128, 128])
    a1T = din("a1T", [128, 128])
    wc = din("wc", [24, 128])
    clinT = din("clinT", [24, B])
    wh = din("wh", [128, 1])
    g1 = din("g1", [F1 // NCORES, 1], f32)   # per-core feature shard
    be1 = din("be1", [F1 // NCORES, 1], f32)
    g2 = din("g2", [128, len(MT2)], f32)
    be2 = din("be2", [128, len(MT2)], f32)
    g3 = din("g3", [128, len(MT3)], f32)
    be3 = din("be3", [128, len(MT3)], f32)
    g4 = din("g4", [F4, 1], f32)
    be4 = din("be4", [F4, 1], f32)
    bc = din("bc", [128, 1], f32)
    bh = din("bh", [1, 1], f32)

    out_v = nc.dram_tensor("out_v", [2, F4, B], f32, kind="ExternalOutput").ap()
    out_h = nc.dram_tensor("out_h", [1, B], f32, kind="ExternalOutput").ap()

    from contextlib import ExitStack

    with tile.TileContext(nc) as tc, ExitStack() as es:
        cpool = es.enter_context(tc.tile_pool(name="c", bufs=1))
        psum = es.enter_context(tc.tile_pool(name="psx", bufs=8, space="PSUM"))
        spool = es.enter_context(tc.tile_pool(name="s", bufs=1))
        opool = es.enter_context(tc.tile_pool(name="o", bufs=6))
        dram = es.enter_context(tc.tile_pool(name="d", bufs=4, space="DRAM"))
        hpool = es.enter_context(tc.tile_pool(name="h", bufs=1))

        # ---- constants to SBUF (tiny) ----
        def cload(ap_in, P, W_, dt, name):
            t = cpool.tile([P, W_], dt, name=f"c_{name}", tag=f"c_{name}")
            nc.sync.dma_start(t[:, :], ap_in)
            return t

        g1_s = cload(g1[:, :], F1 // NCORES, 1, f32, "g1")
        be1_s = cload(be1[:, :], F1 // NCORES, 1, f32, "be1")
        g2_s = cload(g2[:, :], 128, len(MT2), f32, "g2")
        be2_s = cload(be2[:, :], 128, len(MT2), f32, "be2")
        g3_s = cload(g3[:, :], 128, len(MT3), f32, "g3")
        be3_s = cload(be3[:, :], 128, len(MT3), f32, "be3")
        g4_s = cload(g4[:, :], F4, 1, f32, "g4")
        be4_s = cload(be4[:, :], F4, 1, f32, "be4")
        bc_s = cload(bc[:, :], 128, 1, f32, "bc")
        bh_s = cload(bh[:, :], 1, 1, f32, "bh")
        wh_s = cload(wh[:, :], 128, 1, bf16, "wh")
        a0T_s = cload(a0T[:, :], 128, 128, bf16, "a0T")
        a1T_s = cload(a1T[:, :], 128, 128, bf16, "a1T")
        wc_s = cload(wc[:, :], 24, 128, bf16, "wc")

        # ---- L1: partial = w1_c.T @ xT_c; batch-chunked ReduceScatter ----
        F1SH = F1 // NCORES
        y1_s = hpool.tile([F1SH, B], bf16, name="y1_s")
        stats1 = spool.tile([F1SH, NCH * 6], f32, name="stats1")
        h1c = hpool.tile([F1SH, B], bf16, name="h1c")
        ago = dram.tile([F1, B], bf16, name="ago", tag="ago", bufs=1,
                        addr_space="Shared")
        with tc.tile_pool(name="p1", bufs=1) as p1:
            w1_s = p1.tile([128, len(KC1) * F1], bf16, name="w1_s")
            xb0 = p1.tile([128, len(KC1) * 512], bf16, name="xb", tag="xb",
                          bufs=2)
            # first chunks split into strips so several DMA queues fill them
            for ki, (k0, kn) in enumerate(KC1):
                if ki < 4:
                    for q in range(4):
                        nc.sync.dma_start(
                            w1_s[:kn, ki * F1 + q * 200: ki * F1 + (q + 1) * 200],
                            w1[k0:k0 + kn, q * 200:(q + 1) * 200])
                    nc.sync.dma_start(xb0[:kn, ki * 512:ki * 512 + 256],
                                      xT[k0:k0 + kn, 0:256])
                    nc.sync.dma_start(xb0[:kn, ki * 512 + 256:(ki + 1) * 512],
                                      xT[k0:k0 + kn, 256:512])
                else:
                    nc.sync.dma_start(w1_s[:kn, ki * F1:(ki + 1) * F1],
                                      w1[k0:k0 + kn, :])
                    nc.sync.dma_start(xb0[:kn, ki * 512:(ki + 1) * 512],
                                      xT[k0:k0 + kn, 0:512])
            for j in range(NCH):
                if j == 0:
                    xb = xb0
                else:
                    xb = p1.tile([128, len(KC1) * 512], bf16, name="xb",
                                 tag="xb", bufs=2)
                    for ki, (k0, kn) in enumerate(KC1):
                        nc.sync.dma_start(
                            xb[:kn, ki * 512:(ki + 1) * 512],
                            xT[k0:k0 + kn, j * 512:(j + 1) * 512])
                part = dram.tile([F1, 512], bf16, name=f"part{j}", tag="part",
                                 bufs=4)
                for mi, (m0, mn) in enumerate(MT1):
                    ps = psum.tile([128, 512], f32, name=f"ps1_{j}_{mi}",
                                   tag="ps")
                    for ki, (k0, kn) in enumerate(KC1):
                        nc.tensor.matmul(
                            ps[:mn, :],
                            w1_s[:kn, ki * F1 + m0: ki * F1 + m0 + mn],
                            xb[:kn, ki * 512:(ki + 1) * 512],
                            start=(ki == 0), stop=(ki == len(KC1) - 1))
                    ob = opool.tile([128, 512], bf16, name="ob", tag="ob")
                    nc.vector.tensor_copy(ob[:mn, :], ps[:mn, :])
                    nc.sync.dma_start(part[m0:m0 + mn, :], ob[:mn, :])
                rs = dram.tile([F1SH, 512], bf16, name=f"rs{j}", tag="rs",
                               bufs=4)
                nc.gpsimd.collective_compute(
                    "ReduceScatter", ALU.add, replica_groups=RG,
                    ins=[part[:, :].opt()], outs=[rs[:, :].opt()])
                nc.sync.dma_start(y1_s[:, j * 512:(j + 1) * 512], rs[:, :])
                nc.vector.bn_stats(stats1[:, j * 6:(j + 1) * 6],
                                   y1_s[:, j * 512:(j + 1) * 512])

            # BN1 + sigmoid on own shard, then one AllGather of h1
            mv1 = spool.tile([F1SH, 2], f32, name="mv1")
            nc.vector.bn_aggr(mv1[:, :], stats1[:, :])
            sc1, bi1 = _bn_coeffs(nc, spool, mv1[:, 0:1], mv1[:, 1:2],
                                  g1_s[:, 0:1], be1_s[:, 0:1], F1SH, "bn1")
            agi = dram.tile([F1SH, B], bf16, name="agi", tag="agi", bufs=1)
            for j in range(NCH):
                nc.scalar.activation(h1c[:, j * 512:(j + 1) * 512],
                                     y1_s[:, j * 512:(j + 1) * 512],
                                     AF.Sigmoid, bias=bi1[:, 0:1],
                                     scale=sc1[:, 0:1])
                nc.sync.dma_start(agi[:, j * 512:(j + 1) * 512],
                                  h1c[:, j * 512:(j + 1) * 512])
            nc.gpsimd.collective_compute(
                "AllGather", ALU.bypass, replica_groups=RG,
                ins=[agi[:, :].opt()], outs=[ago[:, :].opt()])

        # ---- clinical path (independent; fills gaps) ----
        clinT_s = hpool.tile([24, B], bf16, name="clinT_s")
        nc.sync.dma_start(clinT_s[:, :], clinT[:, :])
        clinb = hpool.tile([128, B], bf16, name="clinb")
        t1f = hpool.tile([128, B], bf16, name="t1f")
        for n in range(NCH):
            pc = psum.tile([128, 512], f32, name="ps_clin", tag="ps")
            nc.tensor.matmul(pc[:, :], wc_s[:, :],
                             clinT_s[:, n * 512:(n + 1) * 512],
                             start=True, stop=True)
            nc.scalar.activation(clinb[:, n * 512:(n + 1) * 512], pc[:, :],
                                 AF.Sigmoid, bias=bc_s[:, 0:1], scale=1.0)
            pa1 = psum.tile([128, 512], f32, name="ps_a1", tag="ps")
            nc.tensor.matmul(pa1[:, :], a1T_s[:, :],
                             clinb[:, n * 512:(n + 1) * 512],
                             start=True, stop=True)
            nc.scalar.activation(t1f[:, n * 512:(n + 1) * 512], pa1[:, :],
                                 AF.Tanh)
        nc.gpsimd.dma_start(out_v[1, :, :], clinb[:, :])  # bf16 -> f32 cast

        # ---- small-weight prefetch for L2-L4 ----
        w2_s = hpool.tile([128, len(KCF1) * F2P], bf16, name="w2_s")
        for kk, (k0, kn) in enumerate(KCF1):
            nc.sync.dma_start(w2_s[:kn, kk * F2P:(kk + 1) * F2P],
                              w2[k0:k0 + kn, :])
        w3_s = hpool.tile([128, len(MT2) * F3], bf16, name="w3_s")
        for kk, (k0, kn) in enumerate(MT2):
            nc.sync.dma_start(w3_s[:kn, kk * F3:(kk + 1) * F3],
                              w3[k0:k0 + kn, :])
        w4_s = hpool.tile([128, len(KC4) * F4], bf16, name="w4_s")
        for kk, (k0, kn) in enumerate(KC4):
            nc.sync.dma_start(w4_s[:kn, kk * F4:(kk + 1) * F4],
                              w4[k0:k0 + kn, :])

        # ---- h1 assembled from AllGather output, kk-chunked ----
        h1_s = hpool.tile([128, len(KCF1) * B], bf16, name="h1_s")
        for kk, (k0, kn) in enumerate(KCF1):
            nc.sync.dma_start(h1_s[:kn, kk * B:(kk + 1) * B], ago[k0:k0 + kn, :])

        # ---- L2: per-m, kk-inner accumulation; BN applies straight from PSUM --
        h2_s = hpool.tile([128, len(MT2) * B], bf16, name="h2_s")
        stats2 = spool.tile([128, len(MT2), NCH, 6], f32, name="stats2")
        mv2 = spool.tile([128, len(MT2), 2], f32, name="mv2")
        for mi, (m0, mn) in enumerate(MT2):
            pss = [psum.tile([128, 512], f32, name=f"ps2_{mi}_{n}",
                             tag="ps") for n in range(NCH)]
            for kk, (k0, kn) in enumerate(KCF1):
                for n in range(NCH):
                    nc.tensor.matmul(
                        pss[n][:mn, :],
                        w2_s[:kn, kk * F2P + m0: kk * F2P + m0 + mn],
                        h1_s[:kn, kk * B + n * 512: kk * B + (n + 1) * 512],
                        start=(kk == 0), stop=(kk == len(KCF1) - 1))
            for n in range(NCH):
                nc.vector.bn_stats(stats2[:, mi, n, :], pss[n][:, :])
            nc.vector.bn_aggr(mv2[:, mi, :], stats2[:, mi, :, :])
            sc2, bi2 = _bn_coeffs(nc, spool, mv2[:, mi, 0:1],
                                  mv2[:, mi, 1:2], g2_s[:, mi:mi + 1],
                                  be2_s[:, mi:mi + 1], 128, f"bn2_{mi}")
            for n in range(NCH):
                nc.scalar.activation(
                    h2_s[:, mi * B + n * 512: mi * B + (n + 1) * 512],
                    pss[n][:, :], AF.Sigmoid, bias=bi2[:, 0:1],
                    scale=sc2[:, 0:1])

        # ---- L3: kk-outer accumulation; BN applies straight from PSUM ----
        h3_s = hpool.tile([128, len(MT3) * B], bf16, name="h3_s")
        stats3 = spool.tile([128, len(MT3), NCH, 6], f32, name="stats3")
        ps3 = [[psum.tile([128, 512], f32, name=f"ps3_{mi}_{n}", tag="ps")
                for n in range(NCH)] for mi in range(len(MT3))]
        for kk, (k0, kn) in enumerate(MT2):
            for mi, (m0, mn) in enumerate(MT3):
                for n in range(NCH):
                    nc.tensor.matmul(
                        ps3[mi][n][:mn, :],
                        w3_s[:kn, kk * F3 + m0: kk * F3 + m0 + mn],
                        h2_s[:kn, kk * B + n * 512: kk * B + (n + 1) * 512],
                        start=(kk == 0), stop=(kk == len(MT2) - 1))
        mv3 = spool.tile([128, len(MT3), 2], f32, name="mv3")
        for mi, (m0, mn) in enumerate(MT3):
            for n in range(NCH):
                nc.vector.bn_stats(stats3[:, mi, n, :], ps3[mi][n][:, :])
            nc.vector.bn_aggr(mv3[:, mi, :], stats3[:, mi, :, :])
            sc3, bi3 = _bn_coeffs(nc, spool, mv3[:, mi, 0:1],
                                  mv3[:, mi, 1:2], g3_s[:, mi:mi + 1],
                                  be3_s[:, mi:mi + 1], 128, f"bn3_{mi}")
            for n in range(NCH):
                nc.scalar.activation(
                    h3_s[:mn, mi * B + n * 512: mi * B + (n + 1) * 512],
                    ps3[mi][n][:mn, :], AF.Sigmoid, bias=bi3[:mn, 0:1],
                    scale=sc3[:mn, 0:1])

        # ---- L4 + BN4 (apply from PSUM); attention + hazard ----
        stats4 = spool.tile([F4, NCH * 6], f32, name="stats4")
        ps4 = [psum.tile([128, 512], f32, name=f"ps4_{n}", tag="ps")
               for n in range(NCH)]
        for kk, (k0, kn) in enumerate(KC4):
            for n in range(NCH):
                nc.tensor.matmul(
                    ps4[n][:, :],
                    w4_s[:kn, kk * F4:(kk + 1) * F4],
                    h3_s[:kn, kk * B + n * 512: kk * B + (n + 1) * 512],
                    start=(kk == 0), stop=(kk == len(KC4) - 1))
        for n in range(NCH):
            nc.vector.bn_stats(stats4[:, n * 6:(n + 1) * 6], ps4[n][:, :])
        mv4 = spool.tile([F4, 2], f32, name="mv4")
        nc.vector.bn_aggr(mv4[:, :], stats4[:, :])
        sc4, bi4 = _bn_coeffs(nc, spool, mv4[:, 0:1], mv4[:, 1:2],
                              g4_s[:, 0:1], be4_s[:, 0:1], F4, "bn4")
        h4b = hpool.tile([128, B], bf16, name="h4b")
        t0f = hpool.tile([128, B], bf16, name="t0f")
        s0f = hpool.tile([128, B], bf16, name="s0f")
        e = hpool.tile([128, B], bf16, name="e_hc")
        cb = hpool.tile([128, B], bf16, name="cb")
        oh = hpool.tile([1, B], f32, name="oh")
        for n in range(NCH):
            sl = slice(n * 512, (n + 1) * 512)
            nc.scalar.activation(h4b[:, sl], ps4[n][:, :], AF.Sigmoid,
                                 bias=bi4[:, 0:1], scale=sc4[:, 0:1])
            pa0 = psum.tile([128, 512], f32, name="ps_a0", tag="ps")
            nc.tensor.matmul(pa0[:, :], a0T_s[:, :], h4b[:, sl],
                             start=True, stop=True)
            nc.scalar.activation(t0f[:, sl], pa0[:, :], AF.Tanh)
            nc.vector.tensor_tensor(t0f[:, sl], t0f[:, sl], t1f[:, sl],
                                    op=ALU.subtract)
            nc.scalar.activation(s0f[:, sl], t0f[:, sl], AF.Sigmoid)
            nc.vector.tensor_tensor(e[:, sl], h4b[:, sl], clinb[:, sl],
                                    op=ALU.subtract)
            nc.vector.tensor_tensor(e[:, sl], s0f[:, sl], e[:, sl],
                                    op=ALU.mult)
            nc.vector.tensor_tensor(cb[:, sl], clinb[:, sl], e[:, sl],
                                    op=ALU.add)
            ph = psum.tile([1, 512], f32, name="ps_h", tag="ps")
            nc.tensor.matmul(ph[:, :], wh_s[:, 0:1], cb[:, sl],
                             start=True, stop=True)
            nc.vector.tensor_scalar(oh[:, sl], ph[:, :], bh_s[0:1, 0:1],
                                    None, op0=ALU.add)
        nc.gpsimd.dma_start(out_v[0, :, :], h4b[:, :])   # casts bf16 -> f32
        nc.sync.dma_start(out_h[:, :], oh[:, :])

    nc.compile()
    return nc


_NC_CACHE = None


def _get_nc():
    global _NC_CACHE
    if _NC_CACHE is None:
        _NC_CACHE = build()
    return _NC_CACHE


def _pack_cols(vec, ntiles, fill):
    """[N] -> [128, ntiles]; column i = vec[i*128:(i+1)*128] (padded)."""
    out = np.full((ntiles * 128,), fill, np.float32)
    out[:len(vec)] = vec
    return np.ascontiguousarray(out.reshape(ntiles, 128).T)


def kernel(**inputs):
    global LAST_EXEC_NS
    f = np.float32
    bf = ml_dtypes.bfloat16
    mrna = np.asarray(inputs["mrna"], f)
    clin_cat = np.asarray(inputs["clin_cat"])
    clin_cont = np.asarray(inputs["clin_cont"], f)
    W = np.asarray(inputs["W"], f)
    w1f = np.asarray(inputs["w1"], f)
    w2f = np.asarray(inputs["w2"], f)
    w3f = np.asarray(inputs["w3"], f)
    w4f = np.asarray(inputs["w4"], f)
    embs = [np.asarray(inputs[f"emb{i}"], f) for i in range(4)]
    g1f = np.asarray(inputs["g1"], f)
    be1f = np.asarray(inputs["be1"], f)

    w2p = np.zeros((F1, F2P), f)
    w2p[:, :F2] = w2f
    w3p = np.zeros((F2P, F3), f)
    w3p[:F2, :] = w3f
    g2p = np.ones((F2P,), f)
    g2p[:F2] = np.asarray(inputs["g2"], f)
    be2p = np.zeros((F2P,), f)
    be2p[:F2] = np.asarray(inputs["be2"], f)

    clin_in = np.concatenate(
        [embs[i][clin_cat[:, i]] for i in range(4)] + [clin_cont], axis=1)

    shared = {
        "w2": w2p.astype(bf), "w3": w3p.astype(bf), "w4": w4f.astype(bf),
        "a0T": np.ascontiguousarray(W[0].T).astype(bf),
        "a1T": np.ascontiguousarray(W[1].T).astype(bf),
        "wc": np.asarray(inputs["wc"], f).astype(bf),
        "clinT": np.ascontiguousarray(clin_in.T).astype(bf),
        "wh": np.asarray(inputs["wh"], f).reshape(128, 1).astype(bf),
        "g2": _pack_cols(g2p, len(MT2), 1.0),
        "be2": _pack_cols(be2p, len(MT2), 0.0),
        "g3": _pack_cols(np.asarray(inputs["g3"], f), len(MT3), 1.0),
        "be3": _pack_cols(np.asarray(inputs["be3"], f), len(MT3), 0.0),
        "g4": np.asarray(inputs["g4"], f).reshape(-1, 1),
        "be4": np.asarray(inputs["be4"], f).reshape(-1, 1),
        "bc": np.asarray(inputs["bc"], f).reshape(-1, 1),
        "bh": np.asarray(inputs["bh"], f).reshape(1, 1),
    }

    in_maps = []
    for c in range(NCORES):
        m = dict(shared)
        m["xT"] = np.ascontiguousarray(
            mrna[:, c * KSH:(c + 1) * KSH].T).astype(bf)
        m["w1"] = w1f[c * KSH:(c + 1) * KSH, :].astype(bf)
        m["g1"] = g1f[c * 100:(c + 1) * 100].reshape(-1, 1)
        m["be1"] = be1f[c * 100:(c + 1) * 100].reshape(-1, 1)
        in_maps.append(m)

    nc = _get_nc()
    trace = bool(int(os.environ.get("KERNEL_TRACE", "0")))
    if trace:
        _install_trace_hook()
    res = run_bass_kernel_spmd(nc, in_maps, CORE_IDS, trace=trace)
    LAST_EXEC_NS = res.exec_time_ns

    ov = res.results[0]["out_v"].astype(f)      # [2, 128, 2048]
    ohv = res.results[0]["out_h"].astype(f)     # [1, 2048]
    v = np.ascontiguousarray(np.transpose(ov, (2, 0, 1)))[..., None]
    hazard = np.ascontiguousarray(ohv.T)        # [2048, 1]
    return hazard, v


def _install_trace_hook():
    import contextlib
    import ctypes
    import types

    if "antenv.axon_hooks" in sys.modules:
        return
    lib = ctypes.CDLL("/opt/axon/libaxon_pjrt.so")
    hook = None
    if hasattr(lib, "axon_start_nrt_profile"):
        lib.axon_start_nrt_profile.argtypes = [
            ctypes.POINTER(ctypes.c_int64), ctypes.c_size_t]
        lib.axon_start_nrt_profile.restype = ctypes.c_int64
        lib.axon_stop_nrt_profile.argtypes = [ctypes.c_char_p]
        lib.axon_stop_nrt_profile.restype = ctypes.c_int64

        @contextlib.contextmanager
        def hook(output_dir, device_ids):
            import jax
            jax.devices()
            if device_ids:
                ids = (ctypes.c_int64 * len(device_ids))(*device_ids)
                rc = lib.axon_start_nrt_profile(ids, len(device_ids))
            else:
                rc = lib.axon_start_nrt_profile(None, 0)
            if rc != 0:
                raise RuntimeError(f"start_nrt_profile rc={rc}")
            try:
                yield
            finally:
                n = lib.axon_stop_nrt_profile(str(output_dir).encode())
                print(f"profile: {n} files -> {output_dir}", file=sys.stderr)

    mod = types.ModuleType("antenv.axon_hooks")
    mod.get_axon_ntff_profile_hook = lambda: hook
    mod.set_axon_ntff_profile_hook = lambda h: None
    sys.modules["antenv.axon_hooks"] = mod
